# revision 19
# baseline (speedup 1.0000x reference)
"""Haar DWT decoder (2-level inverse, zero details) as a Trainium2 Bass kernel.

out[b, c, j, k] = z[b].reshape(C, 128, 128)[c, j//4, k//4] * 0.25
i.e. a 4x4 nearest-neighbor upsample scaled by 1/4.

Data-parallel over batch: 128 samples -> 16 per core on 8 NeuronCores.

The kernel is pure DMA streaming: per core it reads z and writes 16x the
bytes back out, so exec time ~ output bytes / DMA bandwidth. The measured
steady-state DMA rate is ~433 GB/s solo (SBUF AXI fabric ceiling) and
~358 GB/s when the second NeuronCore on the same HBM stack streams
concurrently.

Design notes (each backed by a measured iteration):

1. bf16 I/O. The correctness tolerance (rel_err < 2e-2) leaves precision
   on the table: z is rounded to bf16 on the host (one rounding, ~0.2%
   relative RMS error; the on-device *0.25 is an exact exponent shift,
   adding no further error), the 16x-expanded output is stored as bf16
   (24 MiB/core instead of 48), and the host upcasts to f32. Halves the
   f32 roofline (137-167 us) to ~60-72 us.

2. Group-of-4 flat layout. Both DRAM tensors are declared FLAT and
   processed in groups of 4 consecutive samples: partition p of a group
   holds the group block's coarse rows 12p..12p+11 (i.e. rows
   12(p%32)..12(p%32)+11 of sample 4g + p//32), so a group LOAD is one
   DMA with 3 KiB contiguous per-partition runs and each group's output
   is stored in 4 slices with 12 KiB contiguous per-partition runs.
   Earlier cuts loaded per sample (768 B runs in bf16): those tiny-run
   loads completed absurdly late (a 96 KiB load's completion semaphore
   fired ~6 us after its trigger) and stalled the early muls, capping
   the ramp at ~350 GB/s.

3. Compute balance tuned to measured bf16 engine rates (per 1536-elem
   slice op: DVE broadcast-mul 1.74 us — kr=4 inner-loop restarts
   dominate; DVE contiguous copy 0.69 us — 2 elem/cycle fast path; ACT
   copy 1.57 us — no bf16 speedup). Per slice: DVE does the mul + two
   jr copies (3.1 us), ACT does one jr copy; all three copies depend
   only on the mul. DVE totals ~50 us, ACT ~25 us — both under the
   ~58 us store stream, so the stream paces. Rejected alternatives,
   measured slower: ACT doing two copies (89 us total, ACT-paced);
   DMA-side height replication via 0-stride read APs (2 KiB descriptor
   runs cut the stream to ~385 GB/s, 84 us total).

4. Ring discipline. HWDGE rings are FIFO, and the Tile scheduler
   reorders same-ring DMAs, so: sync (SP) ring carries the group-0 load
   then ALL stores (stores data-depend on loads' consumers, keeping
   order); the scalar (ACT) ring carries only the group 1-3 loads.
   The first slice is further split per coarse row (sub-slice muls,
   DVE-only copies, 4 KiB-run sub-stores) to get the first store
   packets out ~2 us earlier.
"""

import numpy as np
import ml_dtypes

import concourse.bass as bass
import concourse.mybir as mybir
import concourse.tile as tile
from concourse.bass_utils import run_bass_kernel_spmd

# The walrus build in this container rejects instructions carrying more than
# one sync-wait command (codegen: "Too many sync wait commands" — observed on
# a Drain with 3 waits and a DMACopy with 2). Tile freely attaches several
# waits to one instruction, so after tracing we split the excess onto NOPs
# inserted just before the instruction on the same engine; sequential
# dispatch on one engine makes that equivalent.
_MAX_WAITS = 1


def _split_excess_waits(nc: bass.Bass) -> None:
    for f in nc.m.functions:
        for bb in f.blocks:
            insns = bb.instructions
            # Iterate over a snapshot; mutate the live list via insert.
            for ins in list(insns):
                si = ins.sync_info
                if si is None or not si.on_wait or len(si.on_wait) <= _MAX_WAITS:
                    continue
                waits = list(si.on_wait)
                keep = waits[-_MAX_WAITS:]
                spill = waits[:-_MAX_WAITS]
                pos = insns.index(ins)
                nops = []
                for i in range(0, len(spill), _MAX_WAITS):
                    nop = nc.engines[ins.engine].nop(nofuse=True).ins
                    # nop() appended itself to the current bb; pull it out.
                    cur = nc.cur_bb.bb.instructions
                    assert cur[-1] is nop
                    cur.pop()
                    nop.sync_info = mybir.SyncInfo(
                        on_wait=spill[i : i + _MAX_WAITS], on_update=[]
                    )
                    nops.append(nop)
                insns[pos:pos] = nops
                ins.sync_info = mybir.SyncInfo(
                    on_wait=keep, on_update=list(si.on_update)
                )

# Problem constants (hardcoded: module config out_shape=(3,512,512), levels=2)
BATCH = 128
C = 3
CAH = 128  # coarse-approximation spatial dims
CAW = 128
S = 4      # 2**levels upsample factor
H = 512
W = 512
N_CORES = 8
B_SHARD = BATCH // N_CORES  # 16

NPART = 128
GSAMP = 4                      # samples per group
NGROUP = B_SHARD // GSAMP      # 4
ZS = C * CAH * CAW             # z elems per sample (49152)
OS = C * H * W                 # out elems per sample (786432)
ZG = GSAMP * ZS                # z elems per group
OG = GSAMP * OS                # out elems per group
ZPP = ZG // NPART              # 1536 z elems per partition per group (3 KiB)
OPP = OG // NPART              # 24576 out elems per partition per group
NSLICE = 4                     # store slices per group
SPP = OPP // NSLICE            # 6144 out elems per partition per slice (12 KiB)
ZSP = ZPP // NSLICE            # 384 z elems per partition per slice
U = 3                          # coarse rows per partition per slice

BF16 = mybir.dt.bfloat16
NP_BF16 = ml_dtypes.bfloat16


def _hoist_loads_to_preamble(nc: bass.Bass, loads: list) -> None:
    """Move the input-load DMA triggers from the body block into the entry
    block, just before each issuing engine's preamble Drain. The loads then
    fire ~2 us earlier, overlapping the engine-init + barrier window, and
    their data is resident by the time the body's first mul waits on the
    completion semaphore. Safe because: the loads have no sync waits (first
    users of their tiles), their DMAHW semaphores are zero-initialized by
    the runtime (no later in-kernel clear exists that could wipe the early
    +16), and the SBUF destinations are Tile-arena addresses disjoint from
    anything the preamble writes."""
    f = nc.m.functions[0]
    b0, b1 = f.blocks[0], f.blocks[1]
    for ins in loads:
        si = ins.sync_info
        if si is not None and si.on_wait:
            continue  # unexpected dependency — leave it in the body
        if ins not in b1.instructions:
            continue
        # Insert AFTER the engine's barrier release, just before its branch
        # into the body: inserting before the Drain delays the cross-engine
        # barrier itself (it waits for every engine's pre-barrier stream,
        # including these triggers), which pushed the whole body start out.
        # Post-barrier, the triggers only skip the body block's Tile entry
        # overhead, which is pure gain.
        pos = next(
            (
                i
                for i, x in enumerate(b0.instructions)
                if type(x).__name__ == "InstUnconditionalBranch"
                and x.engine == ins.engine
            ),
            None,
        )
        if pos is None:
            continue
        b1.instructions.remove(ins)
        b0.instructions.insert(pos, ins)


def _build_nc(b_shard: int = B_SHARD) -> bass.Bass:
    assert b_shard == B_SHARD
    nc = bass.Bass("TRN2", target_bir_lowering=False, debug=False)
    # FLAT tensors: a group of 4 consecutive samples is one contiguous
    # block on both sides, so group loads and slice stores are fully
    # contiguous per partition (3 KiB and 12 KiB descriptor runs).
    z = nc.dram_tensor("z", [b_shard * ZS], BF16, kind="ExternalInput").ap()
    out = nc.dram_tensor("out", [b_shard * OS], BF16, kind="ExternalOutput").ap()

    with tile.TileContext(nc) as tc:
        with (
            tc.tile_pool(name="zin", bufs=NGROUP) as zin_pool,
            tc.tile_pool(name="wide", bufs=8) as w_pool,
        ):
            zgs = []
            load_insts = []
            for g in range(NGROUP):
                zg = zin_pool.tile([NPART, ZPP], BF16)
                zgs.append(zg)
                if g == 0:
                    # Split group 0's load three ways so slice 0's z (96 KiB,
                    # split across BOTH rings to halve its completion
                    # latency) finishes first: its completion semaphore
                    # gates the very first mul.
                    zflat = z[0:ZG].rearrange("(p x) -> p x", p=NPART)
                    half = ZSP // 2
                    load_insts.append(
                        nc.sync.dma_start(
                            out=zg[:, :half],
                            in_=zflat[:, :half],
                        ).ins
                    )
                    load_insts.append(
                        nc.scalar.dma_start(
                            out=zg[:, half:ZSP],
                            in_=zflat[:, half:ZSP],
                        ).ins
                    )
                    load_insts.append(
                        nc.scalar.dma_start(
                            out=zg[:, ZSP:],
                            in_=zflat[:, ZSP:ZPP],
                        ).ins
                    )
                else:
                    load_insts.append(
                        nc.scalar.dma_start(
                            out=zg[:],
                            in_=z[g * ZG : (g + 1) * ZG].rearrange(
                                "(p x) -> p x", p=NPART
                            ),
                        ).ins
                    )

            slice_idx = 0
            for g in range(NGROUP):
                og = out[g * OG : (g + 1) * OG].rearrange("(p x) -> p x", p=NPART)
                for t in range(NSLICE):
                    # This slice's 3 coarse rows per partition.
                    zq = zgs[g][:, t * ZSP : (t + 1) * ZSP].rearrange(
                        "p (u kc) -> p u kc", u=U
                    )
                    zb = zq.unsqueeze(3).broadcast_to([NPART, U, CAW, S])

                    w2 = w_pool.tile([NPART, SPP], BF16, tag="wide")
                    w2v = w2[:].rearrange(
                        "p (u jr kc kr) -> p u jr kc kr", u=U, jr=S, kc=CAW, kr=S
                    )
                    w2f = w2[:].rearrange("p (u jr k) -> p u jr k", u=U, jr=S)
                    ost = og[:, t * SPP : (t + 1) * SPP]

                    if g == 0 and t < 2:
                        # Head of the pipeline (first two slices): work per
                        # coarse row u and store each row's expansion as
                        # soon as it's ready (4 KiB runs), DVE-only copies
                        # (283 ns each at this size) — ACT is busy with
                        # load triggers. high_priority pins these ahead in
                        # the scheduler, which otherwise interleaves the
                        # next slice's 1.75 us mul before these copies and
                        # starves the stream during the ramp.
                        with tc.high_priority():
                            for u in range(U):
                                nc.vector.tensor_scalar_mul(
                                    w2v[:, u, 0], zb[:, u], 0.25
                                )
                                for jr in range(1, S):
                                    nc.vector.tensor_copy(
                                        w2f[:, u, jr], w2f[:, u, 0]
                                    )
                                nc.sync.dma_start(
                                    out=ost.rearrange("p (u x) -> p u x", u=U)[:, u],
                                    in_=w2f[:, u].rearrange("p jr x -> p (jr x)"),
                                )
                        slice_idx += 1
                        continue

                    # Width-expand x4 (with the 1/4 scale) via a 0-stride
                    # broadcast input, then replicate jr0 into jr1..3: all
                    # three copies depend only on the mul.
                    nc.vector.tensor_scalar_mul(w2v[:, :, 0], zb, 0.25)
                    nc.scalar.copy(w2f[:, :, 1], w2f[:, :, 0])
                    nc.vector.tensor_copy(w2f[:, :, 2], w2f[:, :, 0])
                    nc.vector.tensor_copy(w2f[:, :, 3], w2f[:, :, 0])

                    # One fully-contiguous 1.5 MiB store per slice, 12 KiB
                    # descriptor runs on both sides. Slices 1-2 stay on the
                    # sync ring (the scalar ring still has load packets in
                    # flight); later slices alternate rings.
                    if slice_idx >= 3 and slice_idx % 2 == 1:
                        nc.scalar.dma_start(out=ost, in_=w2[:])
                    else:
                        nc.sync.dma_start(out=ost, in_=w2[:])
                    slice_idx += 1

    _split_excess_waits(nc)
    _hoist_loads_to_preamble(nc, load_insts)
    return nc


_NC_CACHE: dict[int, bass.Bass] = {}


def _get_nc(b_shard: int = B_SHARD) -> bass.Bass:
    if b_shard not in _NC_CACHE:
        _NC_CACHE[b_shard] = _build_nc(b_shard)
    return _NC_CACHE[b_shard]


def _shard_inputs(z: np.ndarray) -> list[dict[str, np.ndarray]]:
    zb = np.ascontiguousarray(z, dtype=np.float32).astype(NP_BF16)
    return [
        {"z": np.ascontiguousarray(zb[i * B_SHARD : (i + 1) * B_SHARD]).reshape(-1)}
        for i in range(N_CORES)
    ]


def kernel(z: np.ndarray) -> np.ndarray:
    assert z.shape == (BATCH, C * CAH * CAW), z.shape
    nc = _get_nc()
    in_maps = _shard_inputs(z)
    res = run_bass_kernel_spmd(nc, in_maps, list(range(N_CORES)))
    return np.concatenate(
        [
            res.results[i]["out"].astype(np.float32).reshape(B_SHARD, C, H, W)
            for i in range(N_CORES)
        ],
        axis=0,
    )
